# revision 16
# baseline (speedup 1.0000x reference)
"""Trainium2 Bass kernel for an 8-expert MoE FFN layer (nn_MoELayer).

Reference computation (per expert e over its contiguous 1024-token chunk):
    h = gelu(x_e @ w1[e] + b1[e]);  y_e = h @ w2[e] + b2[e]

Sharding: expert parallelism — core e holds expert e's weights and its token
chunk (the gate yields equal contiguous chunks, so no all-to-all is needed).
Each core runs the same SPMD program on its own data.

Per-core kernel (T=1024 tokens, D=1024, F=4096), all matmuls in fp16 with
fp32 PSUM accumulation (216 ns per 512-wide matmul incl. fast weight load —
the PE's measured floor; fp32 would be 4x slower, float32r 5%% slower):
  phase 1: for each 128-wide f-tile: h^T[ft] = gelu(w1[:,ft]^T @ x^T + b1[ft])
           (f on partitions -> b1 is a per-partition ACT bias; h^T resident in SBUF)
  phase 2: for each 128-wide dm-tile: y^T[dmo] = w2[:,dmo]^T @ h^T + b2[dmo]
           (dm-tile outer -> one 2-bank PSUM accumulator at a time)
All layout transposes/repacks are done on the host so every DMA is a large
partition-contiguous stream. A short burst of dummy matmuls on scratch data
warms the PE clock (HAM) while the first input DMAs are in flight.
"""

import os

import numpy as np

# The kernel executes through the axon PJRT backend; a CPU pin (e.g. set for
# a jax reference run) would break NEFF dispatch in this process.
if os.environ.get("JAX_PLATFORMS") == "cpu":
    del os.environ["JAX_PLATFORMS"]

E = 8          # experts == cores
B, S = 2, 4096
D = 1024       # d_model
F = 4096       # d_ff
T = (B * S) // E  # tokens per expert chunk = 1024
P = 128
DO = D // P    # 8  k-tiles of d_model
FT = F // P    # 32 f-tiles of d_ff
DMO = D // P   # 8  output dm-tiles
FT2 = FT // 2  # half-slab of w2 f-tiles
NCHUNK = T // 512  # 2 moving-operand chunks (PSUM bank caps matmul N at 512)
N_WARMUP_MM = 14

_cached = None


def _build():
    import concourse.mybir as mybir
    import concourse.tile as tile
    from concourse import bacc
    from concourse.tile_rust import add_dep_helper

    f32 = mybir.dt.float32
    f16 = mybir.dt.float16

    nc = bacc.Bacc("TRN2", target_bir_lowering=False, debug=False, num_devices=E)

    xT_d = nc.dram_tensor("xT", [NCHUNK, P, DO, 512], f16, kind="ExternalInput")
    w1_d = nc.dram_tensor("w1r", [FT, P, DO, P], f16, kind="ExternalInput")
    bc_d = nc.dram_tensor("bc", [P, FT + DMO], f32, kind="ExternalInput")
    w2_d = nc.dram_tensor("w2r", [DMO, 2, P, FT2, P], f16, kind="ExternalInput")
    yT_d = nc.dram_tensor("yT", [DMO, P, T], f32, kind="ExternalOutput")

    gelu = mybir.ActivationFunctionType.Gelu_apprx_tanh

    with tile.TileContext(nc) as tc:
        with (
            tc.tile_pool(name="xpool", bufs=1) as xpool,
            tc.tile_pool(name="hpool", bufs=1) as hpool,
            tc.tile_pool(name="wpool", bufs=2) as wpool,
            tc.tile_pool(name="cpool", bufs=1) as cpool,
            tc.tile_pool(name="ypool", bufs=2) as ypool,
            tc.tile_pool(name="psum_h", bufs=2, space="PSUM") as psum_h,
            tc.tile_pool(name="psum_y", bufs=2, space="PSUM") as psum_y,
        ):
            # scratch for PE warmup, prepared before anything else queues
            scratch32 = cpool.tile([P, 512], f32)
            nc.gpsimd.memset(scratch32[:], 0.0)
            scratch = cpool.tile([P, 512], f16)
            nc.vector.tensor_copy(scratch[:], scratch32[:])

            # input DMAs in critical-path order: w1[0], x chunk 0, x chunk 1, biases
            w1_tiles = {}
            x_dmas = []
            # contiguous-per-partition destination: [p, c, do*512]
            xT_sb = xpool.tile([P, NCHUNK, DO * 512], f16)
            for ft in range(2):
                w1_tiles[ft] = wpool.tile(
                    [P, DO, P], f16, tag="w1", bufs=4, name="w1_sb"
                )
            # first two w1 tiles ride the scalar HWDGE ring so the sync ring
            # starts streaming x immediately
            nc.scalar.dma_start(w1_tiles[0][:], w1_d.ap()[0])
            nc.scalar.dma_start(w1_tiles[1][:], w1_d.ap()[1])
            x_dmas.append(
                nc.sync.dma_start(
                    xT_sb[:, 0, :], xT_d.ap()[0].rearrange("p do t -> p (do t)")
                )
            )
            w1_tiles[2] = wpool.tile([P, DO, P], f16, tag="w1", bufs=4, name="w1_sb")
            nc.sync.dma_start(w1_tiles[2][:], w1_d.ap()[2])
            x_dmas.append(
                nc.sync.dma_start(
                    xT_sb[:, 1, :], xT_d.ap()[1].rearrange("p do t -> p (do t)")
                )
            )
            bc_sb = cpool.tile([P, FT + DMO], f32)
            nc.sync.dma_start(bc_sb[:], bc_d.ap())
            b1_sb = bc_sb[:, :FT]
            b2_sb = bc_sb[:, FT:]

            # PE warmup: dummy matmuls on scratch while input DMAs stream.
            # Keeps the HAM clock-gate at 2.4 GHz by the time real work lands.
            def warmup_mms(n):
                # parked on the phase-2 psum slots, which are idle here
                for i in range(n):
                    pw = psum_y.tile([P, 512], f32, tag="py", name="pwarm")
                    nc.tensor.matmul(
                        pw[:], scratch[:, :P], scratch[:], start=True, stop=True
                    )

            warmup_mms(N_WARMUP_MM)

            h_sb = hpool.tile([P, FT, T], f16)

            # ---- phase 1: h^T = gelu(w1^T x^T + b1), one 128-row f-tile at a time
            # per-(ft, chunk) 1-bank PSUM tiles; the first three f-tiles run
            # chunk-0 first so the PE streams while x chunk 1 is on the wire
            def mm1_group(ph, w1_sb, c):
                mm = None
                for do in range(DO):
                    mm = nc.tensor.matmul(
                        ph[:],
                        w1_sb[:, do, :],
                        xT_sb[:, c, do * 512 : (do + 1) * 512],
                        start=(do == 0),
                        stop=(do == DO - 1),
                    )
                return mm

            def gelu_chunk(ph, ft, c):
                cs = slice(c * 512, (c + 1) * 512)
                return nc.scalar.activation(
                    h_sb[:, ft, cs], ph[:], gelu, bias=b1_sb[:, ft : ft + 1]
                )

            gelu_insts = {}
            HEAD = 3
            head_ph = {}
            last_c0_mm = None
            for ft in range(HEAD):
                ph = psum_h.tile([P, 512], f32, tag="ph", bufs=4, name="ph")
                head_ph[ft] = ph
                last_c0_mm = mm1_group(ph, w1_tiles[ft], 0)
            # keep the PE clock warm while x chunk 1 finishes streaming;
            # ordered after the head chunk-0 groups so these fill the bubble
            # instead of being hoisted ahead of the real work
            for i in range(8):
                pw = psum_y.tile([P, 512], f32, tag="py", name="pwarm")
                bmm = nc.tensor.matmul(
                    pw[:], scratch[:, :P], scratch[:], start=True, stop=True
                )
                add_dep_helper(
                    bmm.ins,
                    last_c0_mm.ins,
                    sync=False,
                    reason="bubble warmup after head c0 groups",
                )
            for ft in range(HEAD):
                ph = head_ph[ft]
                gelu_insts[(ft, 0)] = gelu_chunk(ph, ft, 0)
                ph2 = psum_h.tile([P, 512], f32, tag="ph", bufs=4, name="ph")
                mm1_group(ph2, w1_tiles[ft], 1)
                gelu_insts[(ft, 1)] = gelu_chunk(ph2, ft, 1)

            for ft in range(HEAD, FT):
                w1_tiles[ft] = wpool.tile(
                    [P, DO, P], f16, tag="w1", bufs=4, name="w1_sb"
                )
                nc.sync.dma_start(w1_tiles[ft][:], w1_d.ap()[ft])
                w1_sb = w1_tiles[ft]
                for c in range(NCHUNK):
                    ph = psum_h.tile([P, 512], f32, tag="ph", bufs=4, name="ph")
                    mm1_group(ph, w1_sb, c)
                    gelu_insts[(ft, c)] = gelu_chunk(ph, ft, c)

            # ---- phase 2: y^T[dmo] = w2[:,dmo]^T h^T + b2[dmo]
            FQ = FT // 4
            for dmo in range(DMO):
                w2_q = []
                for qq in range(4):
                    w2_sb = wpool.tile([P, FQ, P], f16, tag="w2", bufs=4, name="w2_sb")
                    dma = nc.sync.dma_start(
                        w2_sb[:],
                        w2_d.ap()[dmo, qq // 2, :, (qq % 2) * FQ : (qq % 2 + 1) * FQ],
                    )
                    if dmo == 0:
                        # keep dmo 0's prefetch out of the head's w1/xT window
                        add_dep_helper(
                            dma.ins,
                            gelu_insts[(6, 1)].ins,
                            sync=True,
                            reason="delay w2 prefetch past the kernel head",
                        )
                    w2_q.append(w2_sb)
                py = psum_y.tile([P, T], f32, tag="py", name="py")

                def y_flush(c):
                    # bias-add + store in 256 chunks so the DMA overlaps the add
                    cs = slice(c * 256, (c + 1) * 256)
                    y_sb = ypool.tile([P, 256], f32, tag="y", bufs=4, name="y_sb")
                    nc.vector.tensor_scalar_add(
                        y_sb[:], py[:, cs], b2_sb[:, dmo : dmo + 1]
                    )
                    nc.sync.dma_start(yT_d.ap()[dmo, :, cs], y_sb[:])

                if dmo < DMO - 1:
                    for fo in range(FT):
                        wt = w2_q[fo // FQ][:, fo % FQ, :]
                        for c in range(NCHUNK):
                            cs = slice(c * 512, (c + 1) * 512)
                            nc.tensor.matmul(
                                py[:, cs],
                                wt,
                                h_sb[:, fo, cs],
                                start=(fo == 0),
                                stop=(fo == FT - 1),
                            )
                    for c in range(4):
                        y_flush(c)
                else:
                    # last dm-tile: chunk-major so the first 512 columns'
                    # epilogue overlaps the second half's matmuls and only
                    # two small chunks flush after the final matmul. Separate
                    # PSUM tiles per half so Tile doesn't serialize the
                    # first half's reads against the second half's writes.
                    for c in range(NCHUNK):
                        cs = slice(c * 512, (c + 1) * 512)
                        py_c = psum_y.tile([P, 512], f32, tag="py", name="py_c")
                        for fo in range(FT):
                            wt = w2_q[fo // FQ][:, fo % FQ, :]
                            nc.tensor.matmul(
                                py_c[:],
                                wt,
                                h_sb[:, fo, cs],
                                start=(fo == 0),
                                stop=(fo == FT - 1),
                            )
                        for cc in range(2):
                            ccs = slice(cc * 256, (cc + 1) * 256)
                            y_sb = ypool.tile(
                                [P, 256], f32, tag="y", bufs=4, name="y_sb"
                            )
                            nc.vector.tensor_scalar_add(
                                y_sb[:], py_c[:, ccs], b2_sb[:, dmo : dmo + 1]
                            )
                            nc.sync.dma_start(
                                yT_d.ap()[dmo, :, 512 * c + ccs.start : 512 * c + ccs.stop],
                                y_sb[:],
                            )

    nc.compile()
    return nc


def _get_nc():
    global _cached
    if _cached is None:
        _cached = _build()
    return _cached


def make_in_maps(x, w1, b1, w2, b2):
    x = np.asarray(x, dtype=np.float32)
    w1 = np.asarray(w1, dtype=np.float32)
    b1 = np.asarray(b1, dtype=np.float32)
    w2 = np.asarray(w2, dtype=np.float32)
    b2 = np.asarray(b2, dtype=np.float32)

    tokens = x.reshape(E, T, D)
    in_maps = []
    for e in range(E):
        xT = np.ascontiguousarray(
            tokens[e].reshape(NCHUNK, 512, DO, P).transpose(0, 3, 2, 1)
        ).astype(np.float16)  # [c, p, do, t']
        w1r = np.ascontiguousarray(
            w1[e].reshape(DO, P, FT, P).transpose(2, 1, 0, 3)
        ).astype(np.float16)  # [ft, p, do, j]
        bc = np.ascontiguousarray(
            np.concatenate([b1[e].reshape(FT, P).T, b2[e].reshape(DMO, P).T], axis=1)
        )  # [p, ft..dmo]
        w2r = np.ascontiguousarray(
            w2[e].reshape(2, FT2, P, DMO, P).transpose(3, 0, 2, 1, 4)
        ).astype(np.float16)  # [dmo, half, p, fo, j]
        in_maps.append({"xT": xT, "w1r": w1r, "bc": bc, "w2r": w2r})
    return in_maps


def gather_out(results):
    out = np.empty((E, T, D), dtype=np.float32)
    for e in range(E):
        yT = results[e]["yT"]  # [dmo, p, t]
        out[e] = yT.transpose(2, 0, 1).reshape(T, D)
    return out.reshape(B, S, D)


def kernel(x, w1, b1, w2, b2):
    from concourse.bass_utils import run_bass_kernel_spmd

    nc = _get_nc()
    in_maps = make_in_maps(x, w1, b1, w2, b2)
    res = run_bass_kernel_spmd(nc, in_maps, core_ids=list(range(E)))
    return gather_out(res.results)
